# revision 14
# baseline (speedup 1.0000x reference)
"""DiT-like dense transformer on 8 trn2 NeuronCores.

Sharding: DP=2 over batch x TP=4 over heads / MLP hidden.
Layout: feature-major activations [C on partitions, tokens on free] -> no
transposes anywhere. Cross-partition LN stats + partition-broadcasts done
with tiny matmuls (ones / selector lhsT). Softmax without max-subtraction
(QK-norm bounds scores to |s|<=8); denominator via ones-column in V.
Matmuls in bf16, accumulation fp32, AllReduce in fp32.
"""

import os
from contextlib import ExitStack
import numpy as np
import ml_dtypes

import concourse.bass as bass
import concourse.mybir as mybir
import concourse.tile as tile
from concourse import bacc
from concourse.bass import ts, ds
from concourse import bass_utils

F32 = mybir.dt.float32
BF16 = mybir.dt.bfloat16
AF = mybir.ActivationFunctionType

NB, C, H, DH, L, BS = 4, 1024, 16, 64, 1024, 2
TP, NCORES = 4, 8
HID = 4096
P = 128
HPC = H // TP          # heads per core = 4
CS = HPC * DH          # per-core qkv slice = 256
HS = HID // TP         # per-core hidden slice = 1024

LAST_EXEC_NS = None
LAST_RESULTS = None


# ---------------- host-side reference-matching pos embed ----------------
def _sincos_1d(d, pos):
    omega = np.arange(d // 2, dtype=np.float64) / (d / 2.0)
    omega = 1.0 / 10000 ** omega
    out = np.einsum('m,d->md', pos.reshape(-1), omega)
    return np.concatenate([np.sin(out), np.cos(out)], axis=1)


def _sincos_2d(embed_dim, h, w):
    gh = np.arange(h, dtype=np.float32)
    gw = np.arange(w, dtype=np.float32)
    grid = np.stack(np.meshgrid(gw, gh), axis=0).reshape(2, 1, h, w)
    emb = np.concatenate([_sincos_1d(embed_dim // 2, grid[0]),
                          _sincos_1d(embed_dim // 2, grid[1])], axis=1)
    return emb.astype(np.float32)  # (h*w, embed_dim)


def _bf(a):
    return np.ascontiguousarray(a).astype(ml_dtypes.bfloat16)


def _f32(a):
    return np.ascontiguousarray(a).astype(np.float32)


# ---------------- host-side sharding / weight prep ----------------
def _prep_core_inputs(core, inputs, pos):
    g, r = core // TP, core % TP
    x = np.asarray(inputs['x'])[g]                      # [L, C]
    d = {}
    d['x_in'] = _f32(x.T.reshape(8, P, L))
    d['pos_in'] = _f32(pos.T.reshape(8, P, L))

    wqk = np.zeros((NB, 8, P, 2 * CS), np.float32)
    bqk = np.zeros((NB, P, 4), np.float32)
    wv = np.zeros((NB, 8, P, CS), np.float32)
    wproj = np.zeros((NB, 2, P, C), np.float32)
    bproj = np.zeros((NB, P, 8), np.float32)
    wfc1 = np.zeros((NB, 8, P, HS), np.float32)
    bfc1 = np.zeros((NB, P, 8), np.float32)
    wfc2 = np.zeros((NB, 8, P, C), np.float32)
    bfc2 = np.zeros((NB, P, 8), np.float32)
    qk_gb = np.zeros((NB, P, 4), np.float32)

    for i in range(NB):
        A = np.asarray(inputs['qkv_w'][i])              # [C, 3C]
        g1 = np.asarray(inputs['n1_g'][i]); b1 = np.asarray(inputs['n1_b'][i])
        Af = A * g1[:, None]
        bful = b1 @ A + np.asarray(inputs['qkv_b'][i])
        qcols = np.arange(CS * r, CS * r + CS)
        kcols = C + qcols
        vcols = 2 * C + qcols
        cols = np.concatenate([qcols, kcols])
        wqk[i] = Af[:, cols].reshape(8, P, 2 * CS)
        bqk[i] = bful[cols].reshape(4, P).T
        wv[i] = Af[:, vcols].reshape(8, P, CS)
        assert np.allclose(bful[vcols], 0.0, atol=1e-6), "nonzero v bias unsupported"

        wproj[i] = np.asarray(inputs['proj_w'][i])[CS * r: CS * r + CS, :].reshape(2, P, C)
        bproj[i] = (np.asarray(inputs['proj_b'][i]) / TP).reshape(8, P).T

        A2 = np.asarray(inputs['fc1_w'][i])
        g2 = np.asarray(inputs['n2_g'][i]); b2 = np.asarray(inputs['n2_b'][i])
        A2f = A2 * g2[:, None]
        b2ful = b2 @ A2 + np.asarray(inputs['fc1_b'][i])
        hcols = np.arange(HS * r, HS * r + HS)
        wfc1[i] = A2f[:, hcols].reshape(8, P, HS)
        bfc1[i] = b2ful[hcols].reshape(8, P).T
        wfc2[i] = np.asarray(inputs['fc2_w'][i])[hcols, :].reshape(8, P, C)
        bfc2[i] = (np.asarray(inputs['fc2_b'][i]) / TP).reshape(8, P).T

        qk_gb[i, :, 0] = np.tile(np.asarray(inputs['nq_g'][i]), 2)
        qk_gb[i, :, 1] = np.tile(np.asarray(inputs['nq_b'][i]), 2)
        qk_gb[i, :, 2] = np.tile(np.asarray(inputs['nk_g'][i]), 2)
        qk_gb[i, :, 3] = np.tile(np.asarray(inputs['nk_b'][i]), 2)

    d['wqk'] = _bf(wqk); d['bqk'] = _f32(bqk)
    d['wv'] = _bf(wv)
    d['wproj'] = _bf(wproj); d['bproj'] = _f32(bproj)
    d['wfc1'] = _bf(wfc1); d['bfc1'] = _f32(bfc1)
    d['wfc2'] = _bf(wfc2); d['bfc2'] = _f32(bfc2)
    d['qk_gb'] = _f32(qk_gb)

    # final layer: permute cols so feature = c + 64*t  (t = p*2+q)
    finw = np.asarray(inputs['fin_w'])                  # [C, 256]
    finb = np.asarray(inputs['fin_b'])
    perm = np.zeros(256, np.int64)
    for t in range(4):
        for c in range(64):
            perm[c + 64 * t] = 4 * c + t
    d['wfin'] = _bf(finw[:, perm].reshape(8, P, 256))
    d['bfin'] = _f32(finb[perm].reshape(2, P).T)

    c1w = np.asarray(inputs['c1_w'])                    # (16, 64, 3, 3)
    c2w = np.asarray(inputs['c2_w'])                    # (3, 16, 3, 3)
    d['wc1'] = _bf(np.stack([c1w[:, :, t // 3, t % 3].T for t in range(9)]))
    d['bc1'] = _f32(np.asarray(inputs['c1_b']).reshape(16, 1))
    d['wc2'] = _bf(np.stack([c2w[:, :, t // 3, t % 3].T for t in range(9)]))
    d['bc2'] = _f32(np.asarray(inputs['c2_b']).reshape(3, 1))
    d['c_ones'] = _bf(np.ones((P, 1)))
    e2 = np.zeros((P, 2)); e2[:64, 0] = 1; e2[64:, 1] = 1
    d['c_e2'] = _bf(e2)
    sel2 = np.zeros((2, P)); sel2[0, :64] = 1; sel2[1, 64:] = 1
    d['c_sel2'] = _bf(sel2)
    d['c_ones1'] = _bf(np.ones((1, P)))
    return d


# ---------------- device program ----------------
def _build(qk_affine_trivial):
    nc = bacc.Bacc("TRN2", target_bir_lowering=False, debug=False,
                   enable_asserts=False, num_devices=NCORES)

    def din(name, shape, dtype):
        return nc.dram_tensor(name, list(shape), dtype, kind="ExternalInput").ap()

    x_d = din('x_in', (8, P, L), F32)
    pos_d = din('pos_in', (8, P, L), F32)
    wqk_d = din('wqk', (NB, 8, P, 2 * CS), BF16)
    bqk_d = din('bqk', (NB, P, 4), F32)
    wv_d = din('wv', (NB, 8, P, CS), BF16)
    wproj_d = din('wproj', (NB, 2, P, C), BF16)
    bproj_d = din('bproj', (NB, P, 8), F32)
    wfc1_d = din('wfc1', (NB, 8, P, HS), BF16)
    bfc1_d = din('bfc1', (NB, P, 8), F32)
    wfc2_d = din('wfc2', (NB, 8, P, C), BF16)
    bfc2_d = din('bfc2', (NB, P, 8), F32)
    qkgb_d = din('qk_gb', (NB, P, 4), F32)
    wfin_d = din('wfin', (8, P, 256), BF16)
    bfin_d = din('bfin', (P, 2), F32)
    wc1_d = din('wc1', (9, 64, 16), BF16)
    bc1_d = din('bc1', (16, 1), F32)
    wc2_d = din('wc2', (9, 16, 3), BF16)
    bc2_d = din('bc2', (3, 1), F32)
    onesbf_d = din('c_ones', (P, 1), BF16)
    e2_d = din('c_e2', (P, 2), BF16)
    sel2_d = din('c_sel2', (2, P), BF16)
    ones1_d = din('c_ones1', (1, P), BF16)
    y_d = nc.dram_tensor('y', [3, 64, 64], F32, kind="ExternalOutput").ap()

    RG = [[0, 1, 2, 3], [4, 5, 6, 7]]

    with tile.TileContext(nc) as tc:
        with (
            tc.tile_pool(name="const", bufs=1) as cpool,
            tc.tile_pool(name="persist", bufs=1) as persist,
            tc.tile_pool(name="xln", bufs=1) as xlnp,
            tc.tile_pool(name="tmp", bufs=3) as tmp,
            tc.tile_pool(name="rows", bufs=2) as rows,
            tc.tile_pool(name="psum", bufs=2, space="PSUM") as psum,
            tc.tile_pool(name="dram", bufs=2, space="DRAM") as dram,
        ):
            # constants (host-provided; partition-offset memsets are illegal)
            ones_bf = cpool.tile([P, 1], BF16, name="ones_bf")
            nc.sync.dma_start(ones_bf[:], onesbf_d)
            e2 = cpool.tile([P, 2], BF16, name="e2")
            nc.sync.dma_start(e2[:], e2_d)
            sel2 = cpool.tile([2, P], BF16, name="sel2")
            nc.sync.dma_start(sel2[:], sel2_d)
            ones1 = cpool.tile([1, P], BF16, name="ones1")
            nc.sync.dma_start(ones1[:], ones1_d)
            eps5 = cpool.tile([2, 1], F32, name="eps5")
            nc.vector.memset(eps5[:], 1e-5)
            eps6 = cpool.tile([1, 1], F32, name="eps6")
            nc.vector.memset(eps6[:], 1e-6)

            # small dram-resident params -> sbuf once
            bqk_t = cpool.tile([P, NB, 4], F32, name="bqk_t")
            nc.sync.dma_start(bqk_t[:], bqk_d.rearrange("b p f -> p b f"))
            bproj_t = cpool.tile([P, NB, 8], F32, name="bproj_t")
            nc.sync.dma_start(bproj_t[:], bproj_d.rearrange("b p f -> p b f"))
            bfc1_t = cpool.tile([P, NB, 8], F32, name="bfc1_t")
            nc.sync.dma_start(bfc1_t[:], bfc1_d.rearrange("b p f -> p b f"))
            bfc2_t = cpool.tile([P, NB, 8], F32, name="bfc2_t")
            nc.sync.dma_start(bfc2_t[:], bfc2_d.rearrange("b p f -> p b f"))
            qkgb_t = cpool.tile([P, NB, 4], F32, name="qkgb_t")
            nc.sync.dma_start(qkgb_t[:], qkgb_d.rearrange("b p f -> p b f"))
            bfin_t = cpool.tile([P, 2], F32, name="bfin_t")
            nc.sync.dma_start(bfin_t[:], bfin_d)
            wc1_t = cpool.tile([64, 9, 16], BF16, name="wc1_t")
            nc.sync.dma_start(wc1_t[:], wc1_d.rearrange("t p f -> p t f"))
            bc1_t = cpool.tile([16, 1], F32, name="bc1_t")
            nc.sync.dma_start(bc1_t[:], bc1_d)
            wc2_t = cpool.tile([16, 9, 3], BF16, name="wc2_t")
            nc.sync.dma_start(wc2_t[:], wc2_d.rearrange("t p f -> p t f"))
            bc2_t = cpool.tile([3, 1], F32, name="bc2_t")
            nc.sync.dma_start(bc2_t[:], bc2_d)
            wfin_t = cpool.tile([P, 8, 256], BF16, name="wfin_t")
            nc.sync.dma_start(wfin_t[:], wfin_d.rearrange("c p f -> p c f"))

            # residual stream (feature-major) = x + pos
            x_res = persist.tile([P, 8, L], F32, name="x_res")
            nc.sync.dma_start(x_res[:], x_d.rearrange("c p t -> p c t"))
            for ci in range(8):
                pt = tmp.tile([P, L], F32, name="pt", tag="posc", bufs=2)
                nc.sync.dma_start(pt[:], pos_d[ci])
                nc.vector.tensor_add(x_res[:, ci], x_res[:, ci], pt[:])

            vaug = persist.tile([P, 8, HPC, DH + 1], BF16, name="vaug")
            nc.vector.memset(vaug[:, :, :, DH:DH + 1], 1.0)
            q_all = persist.tile([P, 2, L], BF16, name="q_all")
            k_all = persist.tile([P, 2, L], BF16, name="k_all")
            o_all = persist.tile([P, 2, L], BF16, name="o_all")
            gel = persist.tile([P, 8, L], BF16, name="gel")
            fsb = persist.tile([P, 8, L], BF16, name="fsb")

            def layernorm(src, dst, eps_ap):
                """src [P,8,L] f32 feature-major -> dst [P,8,L] bf16 normalized
                over the 1024 features (partitions x 8 chunks)."""
                for t2 in range(2):
                    tok = ts(t2, 512)
                    psum_s = psum.tile([1, 512], F32, tag="stat", name="ln_s")
                    psum_q = psum.tile([1, 512], F32, tag="stat", name="ln_q")
                    for ci in range(8):
                        xb = tmp.tile([P, 512], BF16, tag="xb", name="xb", bufs=2)
                        nc.scalar.activation(xb[:], src[:, ci, tok], AF.Identity)
                        xq = tmp.tile([P, 512], BF16, tag="xq", name="xq", bufs=2)
                        nc.scalar.activation(xq[:], src[:, ci, tok], AF.Square)
                        nc.tensor.matmul(psum_s[:], ones_bf[:], xb[:],
                                         start=(ci == 0), stop=(ci == 7))
                        nc.tensor.matmul(psum_q[:], ones_bf[:], xq[:],
                                         start=(ci == 0), stop=(ci == 7))
                    mu = rows.tile([1, 512], F32, tag="r1", name="mu")
                    nc.vector.tensor_scalar_mul(mu[:], psum_s[:], 1.0 / C)
                    ex2 = rows.tile([1, 512], F32, tag="r2", name="ex2")
                    nc.vector.tensor_scalar_mul(ex2[:], psum_q[:], 1.0 / C)
                    var = rows.tile([1, 512], F32, tag="r3", name="var")
                    nc.vector.tensor_mul(var[:], mu[:], mu[:])
                    nc.vector.tensor_sub(var[:], ex2[:], var[:])
                    std = rows.tile([1, 512], F32, tag="r4", name="std")
                    nc.scalar.activation(std[:], var[:], AF.Sqrt, bias=eps_ap)
                    rs = rows.tile([1, 512], F32, tag="r5", name="rs")
                    nc.vector.reciprocal(rs[:], std[:])
                    rbf = rows.tile([1, 2, 512], BF16, tag="r6", name="rbf")
                    nc.vector.tensor_copy(rbf[:, 0], rs[:])
                    nc.vector.tensor_mul(rbf[:, 1], mu[:], rs[:])
                    p_rs = psum.tile([P, 512], F32, tag="bc", name="p_rs")
                    nc.tensor.matmul(p_rs[:], ones1[:], rbf[:, 0])
                    p_mr = psum.tile([P, 512], F32, tag="bc", name="p_mr")
                    nc.tensor.matmul(p_mr[:], ones1[:], rbf[:, 1])
                    for ci in range(8):
                        tt = tmp.tile([P, 512], BF16, tag="lnt", name="lnt")
                        nc.vector.tensor_mul(tt[:], src[:, ci, tok], p_rs[:])
                        nc.vector.tensor_sub(dst[:, ci, tok], tt[:], p_mr[:])

            wctx = ExitStack()
            wts = wctx.enter_context(tc.tile_pool(name="wts", bufs=1))
            wts2 = wctx.enter_context(tc.tile_pool(name="wts2", bufs=1))
            for i in range(NB):
                wqk_t = wts2.tile([P, 8, 2 * CS], BF16, tag="wqk", name="wqk_t")
                nc.sync.dma_start(wqk_t[:], wqk_d[i].rearrange("c p f -> p c f"))
                wv_t = wts2.tile([P, 8, CS], BF16, tag="wv", name="wv_t")
                nc.sync.dma_start(wv_t[:], wv_d[i].rearrange("c p f -> p c f"))
                wproj_t = wts.tile([P, 2, C], BF16, tag="wproj", name="wproj_t")
                nc.sync.dma_start(wproj_t[:], wproj_d[i].rearrange("c p f -> p c f"))
                wfc1_t = wts.tile([P, 8, HS], BF16, tag="wfc1", name="wfc1_t")
                nc.sync.dma_start(wfc1_t[:], wfc1_d[i].rearrange("c p f -> p c f"))
                wfc2_t = wts.tile([P, 8, C], BF16, tag="wfc2", name="wfc2_t")
                nc.sync.dma_start(wfc2_t[:], wfc2_d[i].rearrange("c p f -> p c f"))

                xln = xlnp.tile([P, 8, L], BF16, tag="xln", name="xln")
                layernorm(x_res, xln, eps5[0:1])

                # ---- q/k projections + per-head LN ----
                for fc in range(4):
                    dst = q_all if fc < 2 else k_all
                    dc = fc % 2
                    for t2 in range(2):
                        tok = ts(t2, 512)
                        pq = psum.tile([P, 512], F32, tag="mm", name="pq")
                        for ci in range(8):
                            nc.tensor.matmul(pq[:], wqk_t[:, ci, ds(fc * P, P)],
                                             xln[:, ci, tok],
                                             start=(ci == 0), stop=(ci == 7))
                        qkb = tmp.tile([P, 512], BF16, tag="qkb", name="qkb")
                        nc.scalar.activation(qkb[:], pq[:], AF.Identity,
                                             bias=bqk_t[:, i, fc:fc + 1])
                        qks = tmp.tile([P, 512], BF16, tag="qks", name="qks", bufs=2)
                        nc.scalar.activation(qks[:], qkb[:], AF.Square)
                        ps_s = psum.tile([2, 512], F32, tag="stat", name="ps_s")
                        nc.tensor.matmul(ps_s[:], e2[:], qkb[:])
                        ps_q = psum.tile([2, 512], F32, tag="stat", name="ps_q")
                        nc.tensor.matmul(ps_q[:], e2[:], qks[:])
                        mu = rows.tile([2, 512], F32, tag="r1", name="qmu")
                        nc.vector.tensor_scalar_mul(mu[:], ps_s[:], 1.0 / DH)
                        ex2 = rows.tile([2, 512], F32, tag="r2", name="qex2")
                        nc.vector.tensor_scalar_mul(ex2[:], ps_q[:], 1.0 / DH)
                        var = rows.tile([2, 512], F32, tag="r3", name="qvar")
                        nc.vector.tensor_mul(var[:], mu[:], mu[:])
                        nc.vector.tensor_sub(var[:], ex2[:], var[:])
                        std = rows.tile([2, 512], F32, tag="r4", name="qstd")
                        nc.scalar.activation(std[:], var[:], AF.Sqrt, bias=eps5[:])
                        rs = rows.tile([2, 512], F32, tag="r5", name="qrs")
                        nc.vector.reciprocal(rs[:], std[:])
                        rsb = rows.tile([2, 512], BF16, tag="r6", name="qrsb")
                        nc.vector.tensor_copy(rsb[:], rs[:])
                        mrb = rows.tile([2, 512], BF16, tag="r7", name="qmrb")
                        nc.vector.tensor_mul(mrb[:], mu[:], rs[:])
                        p_rs = psum.tile([P, 512], F32, tag="bc", name="qp_rs")
                        nc.tensor.matmul(p_rs[:], sel2[:], rsb[:])
                        p_mr = psum.tile([P, 512], F32, tag="bc", name="qp_mr")
                        nc.tensor.matmul(p_mr[:], sel2[:], mrb[:])
                        tt = tmp.tile([P, 512], BF16, tag="lnt", name="qtt")
                        nc.vector.tensor_mul(tt[:], qkb[:], p_rs[:])
                        if qk_affine_trivial:
                            nc.vector.tensor_sub(dst[:, dc, tok], tt[:], p_mr[:])
                        else:
                            tt2 = tmp.tile([P, 512], BF16, tag="lnt2", name="qtt2")
                            nc.vector.tensor_sub(tt2[:], tt[:], p_mr[:])
                            go = 0 if fc < 2 else 2
                            nc.vector.tensor_scalar(
                                dst[:, dc, tok], tt2[:],
                                qkgb_t[:, i, go:go + 1], qkgb_t[:, i, go + 1:go + 2],
                                mybir.AluOpType.mult, mybir.AluOpType.add)

                # ---- v projection (token-major) ----
                for t8 in range(8):
                    pv = psum.tile([P, CS], F32, tag="mm", name="pv")
                    for ci in range(8):
                        nc.tensor.matmul(pv[:], xln[:, ci, ts(t8, P)],
                                         wv_t[:, ci, :],
                                         start=(ci == 0), stop=(ci == 7))
                    nc.scalar.activation(
                        vaug[:, t8, :, 0:DH],
                        pv[:].rearrange("p (h d) -> p h d", h=HPC),
                        AF.Identity)

                # ---- attention ----
                for h in range(HPC):
                    hc, ho = h // 2, (h % 2) * DH
                    for qc in range(2):
                        tok = ts(qc, 512)
                        po = psum.tile([P, 512], F32, tag="po", name="po")
                        for kc in range(8):
                            pscr = psum.tile([P, 512], F32, tag="mm", name="pscr")
                            nc.tensor.matmul(pscr[:],
                                             k_all[ho:ho + DH, hc, ts(kc, P)],
                                             q_all[ho:ho + DH, hc, tok])
                            ex = tmp.tile([P, 512], BF16, tag="ex", name="ex")
                            nc.scalar.activation(ex[:], pscr[:], AF.Exp,
                                                 scale=1.0 / np.sqrt(DH))
                            nc.tensor.matmul(po[:DH + 1], vaug[:, kc, h, :], ex[:],
                                             start=(kc == 0), stop=(kc == 7))
                        rden = rows.tile([1, 512], F32, tag="r1", name="rden")
                        nc.vector.reciprocal(rden[:], po[DH:DH + 1, :])
                        rdb = rows.tile([1, 512], BF16, tag="r2", name="rdb")
                        nc.vector.tensor_copy(rdb[:], rden[:])
                        pb = psum.tile([DH, 512], F32, tag="bc", name="pb")
                        nc.tensor.matmul(pb[:], ones1[0:1, 0:DH], rdb[:])
                        ou = tmp.tile([DH, 512], BF16, tag="ou", name="ou")
                        nc.scalar.activation(ou[:], po[0:DH, :], AF.Identity)
                        nc.vector.tensor_mul(o_all[ho:ho + DH, hc, tok], ou[:], pb[:])

                # ---- proj + AllReduce + residual ----
                for oc in range(8):
                    for t2 in range(2):
                        pp = psum.tile([P, 512], F32, tag="mm", name="pp")
                        for hc2 in range(2):
                            nc.tensor.matmul(pp[:], wproj_t[:, hc2, ds(oc * P, P)],
                                             o_all[:, hc2, ts(t2, 512)],
                                             start=(hc2 == 0), stop=(hc2 == 1))
                        nc.scalar.activation(fsb[:, oc, ts(t2, 512)], pp[:],
                                             AF.Identity, bias=bproj_t[:, i, oc:oc + 1])
                ar_in = dram.tile([C, L], BF16, tag="arin", name="ar_in")
                ar_out = dram.tile([C, L], BF16, tag="arout", name="ar_out")
                nc.sync.dma_start(ar_in[:].rearrange("(c p) t -> p c t", p=P), fsb[:])
                nc.gpsimd.collective_compute(
                    "AllReduce", mybir.AluOpType.add, replica_groups=RG,
                    ins=[ar_in[:]], outs=[ar_out[:]])
                nc.sync.dma_start(fsb[:], ar_out[:].rearrange("(c p) t -> p c t", p=P))
                for ci in range(8):
                    nc.vector.tensor_add(x_res[:, ci], x_res[:, ci], fsb[:, ci])

                # ---- MLP ----
                xln2 = xlnp.tile([P, 8, L], BF16, tag="xln", name="xln2")
                layernorm(x_res, xln2, eps5[0:1])
                for hc3 in range(8):
                    for t2 in range(2):
                        pf = psum.tile([P, 512], F32, tag="mm", name="pf")
                        for ci in range(8):
                            nc.tensor.matmul(pf[:], wfc1_t[:, ci, ds(hc3 * P, P)],
                                             xln2[:, ci, ts(t2, 512)],
                                             start=(ci == 0), stop=(ci == 7))
                        nc.scalar.activation(gel[:, hc3, ts(t2, 512)], pf[:],
                                             AF.Gelu, bias=bfc1_t[:, i, hc3:hc3 + 1])
                for oc in range(8):
                    for t2 in range(2):
                        pf2 = psum.tile([P, 512], F32, tag="mm", name="pf2")
                        for ci in range(8):
                            nc.tensor.matmul(pf2[:], wfc2_t[:, ci, ds(oc * P, P)],
                                             gel[:, ci, ts(t2, 512)],
                                             start=(ci == 0), stop=(ci == 7))
                        nc.scalar.activation(fsb[:, oc, ts(t2, 512)], pf2[:],
                                             AF.Identity, bias=bfc2_t[:, i, oc:oc + 1])
                ar_in2 = dram.tile([C, L], BF16, tag="arin", name="ar_in2")
                ar_out2 = dram.tile([C, L], BF16, tag="arout", name="ar_out2")
                nc.sync.dma_start(ar_in2[:].rearrange("(c p) t -> p c t", p=P), fsb[:])
                nc.gpsimd.collective_compute(
                    "AllReduce", mybir.AluOpType.add, replica_groups=RG,
                    ins=[ar_in2[:]], outs=[ar_out2[:]])
                nc.sync.dma_start(fsb[:], ar_out2[:].rearrange("(c p) t -> p c t", p=P))
                for ci in range(8):
                    nc.vector.tensor_add(x_res[:, ci], x_res[:, ci], fsb[:, ci])

            # ---- final layer ----
            wctx.close()
            fctx = ExitStack()
            finp = fctx.enter_context(tc.tile_pool(name="finp", bufs=1))
            xfl = xlnp.tile([P, 8, L], BF16, tag="xln", name="xfl")
            layernorm(x_res, xfl, eps6[0:1])
            xfin = finp.tile([P, 2, L], F32, name="xfin")
            for fc in range(2):
                for t2 in range(2):
                    pn = psum.tile([P, 512], F32, tag="mm", name="pn")
                    for ci in range(8):
                        nc.tensor.matmul(pn[:], wfin_t[:, ci, ds(fc * P, P)],
                                         xfl[:, ci, ts(t2, 512)],
                                         start=(ci == 0), stop=(ci == 7))
                    nc.scalar.activation(xfin[:, fc, ts(t2, 512)], pn[:],
                                         AF.Identity, bias=bfin_t[:, fc:fc + 1])

            # unpatchify directly into padded conv input [64, 66x66] bf16
            ipad = finp.tile([64, 66 * 66], BF16, name="ipad")
            nc.vector.memset(ipad[:], 0.0)
            ipad_v = ipad[:].rearrange("c (y x) -> c y x", x=66)
            interior = ipad_v[:, 1:65, 1:65].rearrange(
                "c (i p) (j q) -> c i p j q", p=2, q=2)
            for t in range(4):
                p_, q_ = t // 2, t % 2
                srcv = xfin[64 * (t % 2): 64 * (t % 2) + 64, t // 2, :]
                nc.vector.tensor_copy(
                    interior[:, :, p_, :, q_],
                    srcv.rearrange("c (i j) -> c i j", j=32))
            cpad = finp.tile([16, 66 * 66], BF16, name="cpad")
            nc.vector.memset(cpad[:], 0.0)
            cpad_v = cpad[:].rearrange("c (y x) -> c y x", x=66)
            for ch in range(8):
                y0 = ch * 8
                pc1 = psum.tile([P, 512], F32, tag="mm", name="pc1")
                for t in range(9):
                    dy, dx = t // 3, t % 3
                    nc.tensor.matmul(pc1[0:16], wc1_t[:, t, :],
                                     ipad_v[:, ds(y0 + dy, 8), ds(dx, 64)],
                                     start=(t == 0), stop=(t == 8))
                nc.scalar.activation(cpad_v[0:16, ds(y0 + 1, 8), 1:65],
                                     pc1[0:16].rearrange("c (y x) -> c y x", x=64),
                                     AF.Relu, bias=bc1_t[:])
            for ch in range(8):
                y0 = ch * 8
                pc2 = psum.tile([P, 512], F32, tag="mm", name="pc2")
                for t in range(9):
                    dy, dx = t // 3, t % 3
                    nc.tensor.matmul(pc2[0:3], wc2_t[:, t, :],
                                     cpad_v[:, ds(y0 + dy, 8), ds(dx, 64)],
                                     start=(t == 0), stop=(t == 8))
                ot = tmp.tile([3, 512], F32, tag="oc", bufs=2, name="ot")
                nc.scalar.activation(ot[:], pc2[0:3], AF.Identity, bias=bc2_t[:])
                nc.sync.dma_start(
                    y_d.rearrange("c y x -> c (y x)")[:, ds(ch * 512, 512)], ot[:])
            fctx.close()

    nc.compile()
    return nc


_CACHED = {}


def kernel(**inputs):
    global LAST_EXEC_NS, LAST_RESULTS
    pos = _sincos_2d(C, 32, 32)
    in_maps = [_prep_core_inputs(c, inputs, pos) for c in range(NCORES)]

    nq = np.asarray(inputs['nq_g']); nqb = np.asarray(inputs['nq_b'])
    nk = np.asarray(inputs['nk_g']); nkb = np.asarray(inputs['nk_b'])
    triv = (np.allclose(nq, 1) and np.allclose(nk, 1)
            and np.allclose(nqb, 0) and np.allclose(nkb, 0))
    key = ('prog', triv)
    if key not in _CACHED:
        _CACHED[key] = _build(triv)
    nc = _CACHED[key]

    results = _run(nc, in_maps)
    out = np.stack([results[0]['y'], results[TP]['y']]).astype(np.float32)
    return out


def _make_runner(nc):
    import jax
    from concourse import bass2jax
    bass2jax.install_neuronx_cc_hook()
    in_names, out_names, out_avals, zero_outs = [], [], [], []
    for alloc in nc.m.functions[0].allocations:
        if not isinstance(alloc, mybir.MemoryLocationSet):
            continue
        name = alloc.memorylocations[0].name
        if alloc.kind == "ExternalInput":
            if nc.partition_id_tensor is None or name != nc.partition_id_tensor.name:
                in_names.append(name)
        elif alloc.kind == "ExternalOutput":
            shape = tuple(alloc.tensor_shape)
            dtype = mybir.dt.np(alloc.dtype)
            out_names.append(name)
            out_avals.append(jax.core.ShapedArray(shape, dtype))
            zero_outs.append(np.zeros(shape, dtype))
    n_params = len(in_names)
    pname = nc.partition_id_tensor.name if nc.partition_id_tensor else None
    all_names = tuple(in_names + out_names + ([pname] if pname else []))

    def _body(*args):
        operands = list(args)
        if pname is not None:
            operands.append(bass2jax.partition_id_tensor())
        outs = bass2jax._bass_exec_p.bind(
            *operands, out_avals=tuple(out_avals), in_names=all_names,
            out_names=tuple(out_names), lowering_input_output_aliases=(),
            sim_require_finite=True, sim_require_nnan=True, nc=nc)
        return tuple(outs)

    devices = jax.devices()[:NCORES]
    mesh = bass2jax.Mesh(np.asarray(devices), ("core",))
    PS = bass2jax.PartitionSpec
    nin = n_params + len(out_names)
    sharded = jax.jit(
        bass2jax.shard_map(_body, mesh=mesh, in_specs=(PS("core"),) * nin,
                           out_specs=(PS("core"),) * len(out_names),
                           check_rep=False),
        donate_argnums=tuple(range(n_params, nin)), keep_unused=True)
    return sharded, in_names, out_names, out_avals, zero_outs, mesh


def _run(nc, in_maps):
    global LAST_EXEC_NS
    import jax
    import time as _time
    from jax.sharding import NamedSharding
    from concourse import bass2jax
    if 'runner' not in _CACHED:
        _CACHED['runner'] = _make_runner(nc)
    sharded, in_names, out_names, out_avals, zero_outs, mesh = _CACHED['runner']
    PS = bass2jax.PartitionSpec
    sh = NamedSharding(mesh, PS("core"))
    concat_in = [
        jax.device_put(
            np.concatenate([in_maps[c][n] for c in range(NCORES)], axis=0), sh)
        for n in in_names]
    jax.block_until_ready(concat_in)

    def one_run():
        zz = [np.zeros((NCORES * z.shape[0], *z.shape[1:]), z.dtype)
              for z in zero_outs]
        t0 = _time.perf_counter()
        out = sharded(*concat_in, *zz)
        jax.block_until_ready(out)
        return _time.perf_counter() - t0, out

    _, out_arrs = one_run()
    iters = int(os.environ.get("KBENCH_ITERS", "0"))
    if iters > 0:
        dts = [one_run()[0] for _ in range(iters)]
        LAST_EXEC_NS = int(min(dts) * 1e9)
    return [
        {n: np.asarray(out_arrs[i]).reshape(NCORES, *out_avals[i].shape)[c]
         for i, n in enumerate(out_names)}
        for c in range(NCORES)]
